# revision 2
# baseline (speedup 1.0000x reference)
"""Akima spline interpolation kernel for Trainium2 (8 NeuronCores, data parallel).

Strategy:
  - The reference output is f(x) = akima_spline(x) for x in [0,1). Write
    f(x) = x + g(x); g is a wiggle of scale ~0.1 that varies on the knot
    scale h=1/255. Host precomputes a piecewise-constant table for g on
    4097 uniform bins of width 1/4096 centered at j/4096 (bin j covers
    [(j-0.5)/4096, (j+0.5)/4096)); each entry is the bin mean of g in
    float16. Measured end-to-end rel L2 error of this model: ~4.0e-3
    (budget 2e-2).
  - Device per tile: rbig = act(copy, x, scale=4096, bias=M) rounds
    4096*x to the nearest integer via the magic constant M = 1.5*2^23
    (exact: 4096 is a power of two, so the single fp32 fma rounds once,
    matching np.rint used to build the table); idx = act(copy, rbig,
    bias=-M, dtype=uint32); one gather_flattened fetches the f16 entry;
    one vector-engine add produces x + g. Scalar engine 2 ops, GPSIMD 1
    gather, vector engine 1 op per tile - the gather is the bottleneck.
  - Sharding: pure data parallel on the leading dim (4 of 32 planes per
    core); the 8KB table is replicated to all partitions of every core.
"""
import base64
import json
import sys

import numpy as np

if "/opt/trn_rl_repo" not in sys.path:
    sys.path.insert(0, "/opt/trn_rl_repo")

NODES = 256
N_CORES = 8
ROWS = 128
COLS = 4 * 1024 * 1024 // ROWS  # per-core shard [128, 32768]
F_TILE = 4096
NSEG = 4096                     # bins per unit; table has NSEG+1 entries
TAB_N = NSEG + 8                # padded table length
MAGIC = float(np.float32(1.5 * 2.0 ** 23))

# ----------------------------------------------------------------------------
# Host-side table construction
# ----------------------------------------------------------------------------


def _akima_slopes_f64(value):
    h = 1.0 / (NODES - 1)
    v = value.astype(np.float64)
    m = (v[1:] - v[:-1]) / h
    m_m1 = 2.0 * m[0] - m[1]
    m_m2 = 2.0 * m_m1 - m[0]
    m_p1 = 2.0 * m[-1] - m[-2]
    m_p2 = 2.0 * m_p1 - m[-1]
    me = np.concatenate([[m_m2, m_m1], m, [m_p1, m_p2]])
    w1 = np.abs(me[3:] - me[2:-1])
    w2 = np.abs(me[1:-2] - me[:-3])
    mi_1 = me[1:-2]
    mi = me[2:-1]
    denom = w1 + w2
    safe = np.where(denom > 0, denom, 1.0)
    return np.where(denom > 0, (w1 * mi_1 + w2 * mi) / safe, 0.5 * (mi_1 + mi))


def _spline_f64(x, value, s):
    h = 1.0 / (NODES - 1)
    v = value.astype(np.float64)
    x = np.clip(x, 0.0, 1.0)
    t = x / h
    idx = np.clip(np.floor(t).astype(np.int64), 0, NODES - 2)
    u = t - idx
    v0 = v[idx]
    v1 = v[idx + 1]
    s0 = s[idx]
    s1 = s[idx + 1]
    u2 = u * u
    u3 = u2 * u
    return ((2 * u3 - 3 * u2 + 1) * v0 + (u3 - 2 * u2 + u) * h * s0
            + (-2 * u3 + 3 * u2) * v1 + (u3 - u2) * h * s1)


def _build_table(value):
    s = _akima_slopes_f64(value)
    # bin j covers x in [(j-0.5)/NSEG, (j+0.5)/NSEG); entry = mean of f(x)-x
    SS = 32
    j = np.arange(NSEG + 1)
    offs = (np.arange(SS) + 0.5) / SS - 0.5
    xs = (j[:, None] + offs[None, :]) / NSEG
    xs = np.clip(xs, 0.0, 1.0 - 1e-12)
    g = _spline_f64(xs.ravel(), value, s).reshape(NSEG + 1, SS) - xs
    tab = np.zeros(TAB_N, dtype=np.float16)
    tab[:NSEG + 1] = g.mean(axis=1).astype(np.float16)
    return tab


# ----------------------------------------------------------------------------
# NKI kernel
# ----------------------------------------------------------------------------


def _make_nki_kernel():
    import neuronxcc.nki.language as nl
    import neuronxcc.nki.isa as nisa

    n_tiles = COLS // F_TILE

    def akima_kernel(inputs):
        x, table = inputs[0], inputs[1]
        out = nl.ndarray(shape=[ROWS, COLS], dtype=nl.float32, buffer=nl.shared_hbm)
        tab_sb = nl.load(table)
        i_p = nl.arange(ROWS)[:, None]
        i_f = nl.arange(F_TILE)[None, :]
        magic_bias = nisa.memset((ROWS, 1), MAGIC, nl.float32)
        neg_magic_bias = nisa.memset((ROWS, 1), -MAGIC, nl.float32)

        # Explicit ping-pong SBUF buffers: without them the allocator's
        # address reuse creates WAR hazards that serialize consecutive tiles.
        def mkbufs():
            return dict(
                rbig=nl.ndarray(shape=[ROWS, F_TILE], dtype=nl.float32, buffer=nl.sbuf),
                idx=nl.ndarray(shape=[ROWS, F_TILE], dtype=nl.uint32, buffer=nl.sbuf),
                g=nl.ndarray(shape=[ROWS, F_TILE], dtype=nl.float16, buffer=nl.sbuf),
            )

        bufs = [mkbufs(), mkbufs()]

        for t in range(n_tiles):
            B = bufs[t % 2]
            sl = slice(t * F_TILE, (t + 1) * F_TILE)
            x_sb = nl.load(x[:, sl])
            B['rbig'][i_p, i_f] = nisa.activation(
                np.copy, x_sb, bias=magic_bias, scale=float(NSEG))
            B['idx'][i_p, i_f] = nisa.activation(
                np.copy, B['rbig'][i_p, i_f], bias=neg_magic_bias, dtype=nl.uint32)
            B['g'][i_p, i_f] = nl.gather_flattened(
                data=tab_sb, indices=B['idx'][i_p, i_f])
            r = nisa.tensor_tensor(B['g'][i_p, i_f], x_sb, np.add,
                                   dtype=nl.float32)
            nl.store(out[:, sl], r)
        return [out]

    return akima_kernel


# ----------------------------------------------------------------------------
# jax integration (AwsNeuronCustomNativeKernel custom call, SPMD over 8 cores)
# ----------------------------------------------------------------------------

_EXEC_CACHE = {}


def _build_executor():
    if "exec" in _EXEC_CACHE:
        return _EXEC_CACHE["exec"]

    import jax
    from jax.interpreters import mlir
    from jax._src.interpreters.mlir import custom_call as _mlir_custom_call
    from jax.sharding import Mesh, PartitionSpec
    from jax.experimental.shard_map import shard_map
    from concourse.nki import raw_nki
    from concourse.bass2jax import install_neuronx_cc_hook

    install_neuronx_cc_hook()

    nki_func = _make_nki_kernel()

    prim = jax.extend.core.Primitive("akima_exec")
    prim.multiple_results = True

    @prim.def_abstract_eval
    def _abs(*_, **__):
        return (jax.core.ShapedArray((ROWS, COLS), np.float32),)

    def _layouts(shapes):
        return [list(reversed(range(len(s)))) for s in shapes]

    def _lowering(ctx, *in_nodes):
        from neuronxcc.starfish.penguin.ir.NativeKernel import KERNEL_VERSION

        result_types = [mlir.aval_to_ir_type(a) for a in ctx.avals_out]
        code = raw_nki(nki_func)(list(ctx.avals_in))
        config = {
            "kernel_version": KERNEL_VERSION,
            "func_literal": code.serialize_ir_string("akima_kernel_ir"),
            "grid": [],
            "func_name": "akima_kernel",
            "has_collectives": False,
            "mac_count": 0,
            "tiled": False,
        }
        dumped = base64.b64encode(json.dumps(config).encode()).decode()
        return _mlir_custom_call(
            "AwsNeuronCustomNativeKernel",
            operands=list(in_nodes),
            result_types=result_types,
            operand_layouts=_layouts(a.shape for a in ctx.avals_in),
            result_layouts=_layouts(a.shape for a in ctx.avals_out),
            backend_config=dumped,
        ).results

    mlir.register_lowering(prim, _lowering, platform="neuron")

    devices = jax.devices()[:N_CORES]
    mesh = Mesh(np.asarray(devices), ("core",))

    def _body(x_shard, tab_shard):
        return prim.bind(x_shard, tab_shard)[0]

    sharded = jax.jit(shard_map(
        _body, mesh=mesh,
        in_specs=(PartitionSpec("core"), PartitionSpec("core")),
        out_specs=PartitionSpec("core"),
        check_rep=False,
    ))

    _EXEC_CACHE["exec"] = sharded
    return sharded


# ----------------------------------------------------------------------------
# Public entry point
# ----------------------------------------------------------------------------


def kernel(input: np.ndarray, value: np.ndarray) -> np.ndarray:
    input = np.ascontiguousarray(np.asarray(input, dtype=np.float32))
    value = np.asarray(value, dtype=np.float32)
    assert input.shape == (32, 1024, 1024), input.shape

    tab = _build_table(value)
    table = np.broadcast_to(tab, (ROWS, TAB_N)).copy()

    sharded = _build_executor()

    # shard on the leading dim: core i gets planes [4i, 4i+4)
    x_global = input.reshape(N_CORES * ROWS, COLS)
    tab_global = np.tile(table, (N_CORES, 1))

    out = sharded(x_global, tab_global)
    return np.asarray(out).reshape(32, 1024, 1024)


if __name__ == "__main__":
    inp = np.load("cache/input.npy")
    val = np.load("cache/value.npy")
    out = kernel(input=inp, value=val)
    exp = np.load("cache/expected.npy")
    err = out.astype(np.float64) - exp.astype(np.float64)
    print("rel_l2:", np.linalg.norm(err) / np.linalg.norm(exp))


# revision 3
# speedup vs baseline: 3.4159x; 3.4159x over previous
"""Akima spline interpolation kernel for Trainium2 (8 NeuronCores, data parallel).

Strategy:
  - The reference output is f(x) = akima_spline(x) for x in [0,1). Host
    fits an L2-optimal straight line to f on each of 511 uniform bins of
    width 1/510 centered at j/510 (bin j covers [(j-0.5)/510,
    (j+0.5)/510)): f(x) ~ A[j] + B[j]*w with w = 510*x - j in [-.5,.5).
    A and B are rounded to bf16 and packed into one uint32 per bin
    (A low half, B high half) - a 512-entry table, which fits the GPSIMD
    gather's 512-element pool-buffer window so each tile needs exactly
    one single-window gather. Measured end-to-end rel L2 error of this
    model (including bf16 rounding): ~5.1e-3 (budget 2e-2).
  - Device per tile: rbig = act(copy, x, scale=510, bias=M) rounds 510*x
    to the nearest integer via the magic constant M = 1.5*2^23; two more
    act copies with bias=-M produce the index as uint32 and as float;
    one gather fetches the packed word; the vector engine computes
    w = 510*x - idxf (scalar_tensor_tensor) and evaluates A + B*w as two
    tensor_tensor ops on stride-2 bf16 views of the gathered word.
    Per tile: scalar 3 ops, vector 3 ops, GPSIMD 1 gather - roughly
    balanced, with the gather the slight bottleneck.
  - Sharding: pure data parallel on the leading dim (4 of 32 planes per
    core); the 2KB table is replicated to all partitions of every core.
"""
import base64
import json
import sys

import numpy as np

if "/opt/trn_rl_repo" not in sys.path:
    sys.path.insert(0, "/opt/trn_rl_repo")

NODES = 256
N_CORES = 8
ROWS = 128
COLS = 4 * 1024 * 1024 // ROWS  # per-core shard [128, 32768]
F_TILE = 2048
NSEG = 510                      # bins per unit; table has NSEG+1 entries
TAB_N = 512                     # padded table length (= pool buffer window)
MAGIC = float(np.float32(1.5 * 2.0 ** 23))

# ----------------------------------------------------------------------------
# Host-side table construction
# ----------------------------------------------------------------------------


def _akima_slopes_f64(value):
    h = 1.0 / (NODES - 1)
    v = value.astype(np.float64)
    m = (v[1:] - v[:-1]) / h
    m_m1 = 2.0 * m[0] - m[1]
    m_m2 = 2.0 * m_m1 - m[0]
    m_p1 = 2.0 * m[-1] - m[-2]
    m_p2 = 2.0 * m_p1 - m[-1]
    me = np.concatenate([[m_m2, m_m1], m, [m_p1, m_p2]])
    w1 = np.abs(me[3:] - me[2:-1])
    w2 = np.abs(me[1:-2] - me[:-3])
    mi_1 = me[1:-2]
    mi = me[2:-1]
    denom = w1 + w2
    safe = np.where(denom > 0, denom, 1.0)
    return np.where(denom > 0, (w1 * mi_1 + w2 * mi) / safe, 0.5 * (mi_1 + mi))


def _spline_f64(x, value, s):
    h = 1.0 / (NODES - 1)
    v = value.astype(np.float64)
    x = np.clip(x, 0.0, 1.0)
    t = x / h
    idx = np.clip(np.floor(t).astype(np.int64), 0, NODES - 2)
    u = t - idx
    v0 = v[idx]
    v1 = v[idx + 1]
    s0 = s[idx]
    s1 = s[idx + 1]
    u2 = u * u
    u3 = u2 * u
    return ((2 * u3 - 3 * u2 + 1) * v0 + (u3 - 2 * u2 + u) * h * s0
            + (-2 * u3 + 3 * u2) * v1 + (u3 - u2) * h * s1)


def _build_table(value):
    import ml_dtypes
    s = _akima_slopes_f64(value)
    SS = 64
    j = np.arange(NSEG + 1)
    offs = (np.arange(SS) + 0.5) / SS - 0.5
    xs = (j[:, None] + offs[None, :]) / NSEG
    xs = np.clip(xs, 0.0, 1.0 - 1e-12)
    f = _spline_f64(xs.ravel(), value, s).reshape(NSEG + 1, SS)
    A = f.mean(axis=1)
    B = 12.0 * (f * offs[None, :]).mean(axis=1)
    Ab = A.astype(ml_dtypes.bfloat16).view(np.uint16).astype(np.uint32)
    Bb = B.astype(ml_dtypes.bfloat16).view(np.uint16).astype(np.uint32)
    tab = np.zeros(TAB_N, dtype=np.uint32)
    tab[:NSEG + 1] = Ab | (Bb << 16)
    return tab


# ----------------------------------------------------------------------------
# NKI kernel
# ----------------------------------------------------------------------------


def _make_nki_kernel():
    import neuronxcc.nki.language as nl
    import neuronxcc.nki.isa as nisa

    n_tiles = COLS // F_TILE

    def akima_kernel(inputs):
        x, table = inputs[0], inputs[1]
        out = nl.ndarray(shape=[ROWS, COLS], dtype=nl.float32, buffer=nl.shared_hbm)
        tab_sb = nl.load(table)
        i_p = nl.arange(ROWS)[:, None]
        i_f = nl.arange(F_TILE)[None, :]
        magic_bias = nisa.memset((ROWS, 1), MAGIC, nl.float32)
        neg_magic_bias = nisa.memset((ROWS, 1), -MAGIC, nl.float32)

        # Explicit ping-pong SBUF buffers: without them the allocator's
        # address reuse creates WAR hazards that serialize consecutive tiles.
        def mkbufs():
            return dict(
                rbig=nl.ndarray(shape=[ROWS, F_TILE], dtype=nl.float32, buffer=nl.sbuf),
                idx=nl.ndarray(shape=[ROWS, F_TILE], dtype=nl.uint32, buffer=nl.sbuf),
                idxf=nl.ndarray(shape=[ROWS, F_TILE], dtype=nl.float32, buffer=nl.sbuf),
                g=nl.ndarray(shape=[ROWS, F_TILE], dtype=nl.uint32, buffer=nl.sbuf),
            )

        bufs = [mkbufs(), mkbufs()]

        for t in range(n_tiles):
            B = bufs[t % 2]
            sl = slice(t * F_TILE, (t + 1) * F_TILE)
            x_sb = nl.load(x[:, sl])
            B['rbig'][i_p, i_f] = nisa.activation(
                np.copy, x_sb, bias=magic_bias, scale=float(NSEG))
            B['idx'][i_p, i_f] = nisa.activation(
                np.copy, B['rbig'][i_p, i_f], bias=neg_magic_bias, dtype=nl.uint32)
            B['idxf'][i_p, i_f] = nisa.activation(
                np.copy, B['rbig'][i_p, i_f], bias=neg_magic_bias)
            B['g'][i_p, i_f] = nl.gather_flattened(
                data=tab_sb, indices=B['idx'][i_p, i_f])
            w = nisa.scalar_tensor_tensor(
                data=x_sb, op0=np.multiply, operand0=float(NSEG),
                op1=np.subtract, operand1=B['idxf'][i_p, i_f])
            gb = B['g'].view(nl.bfloat16)  # [P, 2F]: A at even, B at odd
            m = nisa.tensor_tensor(gb[i_p, i_f * 2 + 1], w, np.multiply,
                                   dtype=nl.float32)
            r = nisa.tensor_tensor(m, gb[i_p, i_f * 2], np.add,
                                   dtype=nl.float32)
            nl.store(out[:, sl], r)
        return [out]

    return akima_kernel


# ----------------------------------------------------------------------------
# jax integration (AwsNeuronCustomNativeKernel custom call, SPMD over 8 cores)
# ----------------------------------------------------------------------------

_EXEC_CACHE = {}


def _build_executor():
    if "exec" in _EXEC_CACHE:
        return _EXEC_CACHE["exec"]

    import jax
    from jax.interpreters import mlir
    from jax._src.interpreters.mlir import custom_call as _mlir_custom_call
    from jax.sharding import Mesh, PartitionSpec
    from jax.experimental.shard_map import shard_map
    from concourse.nki import raw_nki
    from concourse.bass2jax import install_neuronx_cc_hook

    install_neuronx_cc_hook()

    nki_func = _make_nki_kernel()

    prim = jax.extend.core.Primitive("akima_exec")
    prim.multiple_results = True

    @prim.def_abstract_eval
    def _abs(*_, **__):
        return (jax.core.ShapedArray((ROWS, COLS), np.float32),)

    def _layouts(shapes):
        return [list(reversed(range(len(s)))) for s in shapes]

    def _lowering(ctx, *in_nodes):
        from neuronxcc.starfish.penguin.ir.NativeKernel import KERNEL_VERSION

        result_types = [mlir.aval_to_ir_type(a) for a in ctx.avals_out]
        code = raw_nki(nki_func)(list(ctx.avals_in))
        config = {
            "kernel_version": KERNEL_VERSION,
            "func_literal": code.serialize_ir_string("akima_kernel_ir"),
            "grid": [],
            "func_name": "akima_kernel",
            "has_collectives": False,
            "mac_count": 0,
            "tiled": False,
        }
        dumped = base64.b64encode(json.dumps(config).encode()).decode()
        return _mlir_custom_call(
            "AwsNeuronCustomNativeKernel",
            operands=list(in_nodes),
            result_types=result_types,
            operand_layouts=_layouts(a.shape for a in ctx.avals_in),
            result_layouts=_layouts(a.shape for a in ctx.avals_out),
            backend_config=dumped,
        ).results

    mlir.register_lowering(prim, _lowering, platform="neuron")

    devices = jax.devices()[:N_CORES]
    mesh = Mesh(np.asarray(devices), ("core",))

    def _body(x_shard, tab_shard):
        return prim.bind(x_shard, tab_shard)[0]

    sharded = jax.jit(shard_map(
        _body, mesh=mesh,
        in_specs=(PartitionSpec("core"), PartitionSpec("core")),
        out_specs=PartitionSpec("core"),
        check_rep=False,
    ))

    _EXEC_CACHE["exec"] = sharded
    return sharded


# ----------------------------------------------------------------------------
# Public entry point
# ----------------------------------------------------------------------------


def kernel(input: np.ndarray, value: np.ndarray) -> np.ndarray:
    input = np.ascontiguousarray(np.asarray(input, dtype=np.float32))
    value = np.asarray(value, dtype=np.float32)
    assert input.shape == (32, 1024, 1024), input.shape

    tab = _build_table(value)
    table = np.broadcast_to(tab, (ROWS, TAB_N)).copy()

    sharded = _build_executor()

    # shard on the leading dim: core i gets planes [4i, 4i+4)
    x_global = input.reshape(N_CORES * ROWS, COLS)
    tab_global = np.tile(table, (N_CORES, 1))

    out = sharded(x_global, tab_global)
    return np.asarray(out).reshape(32, 1024, 1024)


if __name__ == "__main__":
    inp = np.load("cache/input.npy")
    val = np.load("cache/value.npy")
    out = kernel(input=inp, value=val)
    exp = np.load("cache/expected.npy")
    err = out.astype(np.float64) - exp.astype(np.float64)
    print("rel_l2:", np.linalg.norm(err) / np.linalg.norm(exp))
